# revision 29
# baseline (speedup 1.0000x reference)
# Trainium2 Bass kernel for nn_DecoderWithAttention.
#
# Strategy (data-parallel over batch, per the sharding hint):
#   - Host: sort batch by caption length (desc, stable), shard 4 sequences
#     per core across 8 cores, precompute gather offsets / weight layouts
#     (bf16 casts, transposes, bias folding). Length masking of outputs
#     happens on the host (exact: masked entries are zeroed).
#   - Device (identical SPMD program on 8 cores):
#       prologue: zero-pad images in DRAM, indirect-DMA patch gather,
#                 patch embedding, emb @ W_ih_emb precompute (-> DRAM),
#                 enc load + on-chip transposes, att1 precompute, enc
#                 mean, LSTM h0/c0 init.
#       scan:     64 unrolled steps of attention + gating + LSTMCell.
#   - Matmuls are x-stationary (a [K,4] activation transpose is the
#     stationary operand, weights stream) in bf16 with fp32 PSUM accum;
#     recurrent state and softmax stay fp32.
#
# kernel(**inputs) takes FULL unsharded inputs and returns the same tuple
# as reference.reference().

import numpy as np
import ml_dtypes

import concourse.bass as bass
import concourse.bacc as bacc
import concourse.mybir as mybir
import concourse.tile as tile
from concourse import bass_utils
from concourse.masks import make_identity

BF = mybir.dt.bfloat16
F32 = mybir.dt.float32
I32 = mybir.dt.int32
AF = mybir.ActivationFunctionType
OP = mybir.AluOpType

B, T = 32, 64
E = 14
P = E * E                # 196 attention positions
ENC_D, DEC_D, ATT_D = 2048, 512, 512
PATCH, CH = 32, 3
FLAT = PATCH * PATCH * CH  # 3072
EMB = 256
PAD = PATCH // 2 + PATCH // 4  # 24
IMG = 512
PIMG = IMG + PAD         # 536: low-side pad only (kp in [0, 480))
NCORES = 8
BC = B // NCORES         # 4 sequences per core
NPATCH = BC * T          # 256 patches per core
NPOFF = NPATCH * CH * PATCH // 128  # 192 gather entries per partition

KD = DEC_D // 128        # 4
KE = ENC_D // 128        # 16
KF = FLAT // 128         # 24
G4 = 4 * DEC_D           # 2048
N1 = ATT_D + ENC_D + G4  # 4608 fused group-1 width
HP = BC * P // 2         # 392


def build_program(enable_bias, debug=False):
    nc = bacc.Bacc(trn_type="TRN2")

    # ---- I/O ----
    imgs12 = nc.dram_tensor("imgs12", [CH * BC, IMG, IMG], F32, kind="ExternalInput")
    poff = nc.dram_tensor("poff", [128, NPOFF], I32, kind="ExternalInput")
    enc4 = nc.dram_tensor("enc4", [BC, P, ENC_D], F32, kind="ExternalInput")
    W1 = nc.dram_tensor("W1", [DEC_D, N1], BF, kind="ExternalInput")
    W1b = nc.dram_tensor("W1b", [1, G4], BF, kind="ExternalInput")
    Wx = nc.dram_tensor("Wx", [ENC_D, G4], BF, kind="ExternalInput")
    WpeT = nc.dram_tensor("WpeT", [FLAT, EMB], BF, kind="ExternalInput")
    bpe = nc.dram_tensor("bpe", [128, 2], F32, kind="ExternalInput")
    WencT = nc.dram_tensor("WencT", [ENC_D, ATT_D], BF, kind="ExternalInput")
    battT = nc.dram_tensor("battT", [128, 4], F32, kind="ExternalInput")
    Winit = nc.dram_tensor("Winit", [19 * 128, 2 * DEC_D], BF, kind="ExternalInput")
    Wihe = nc.dram_tensor("Wihe", [EMB, G4], BF, kind="ExternalInput")
    bihh = nc.dram_tensor("bihh", [1, G4], BF, kind="ExternalInput")
    Wfcs = nc.dram_tensor("Wfcs", [DEC_D, 3], BF, kind="ExternalInput")
    bfcs = nc.dram_tensor("bfcs", [1, 3], BF, kind="ExternalInput")
    Wful = nc.dram_tensor("Wful", [128, KD], BF, kind="ExternalInput")

    alph_o = nc.dram_tensor("alph_o", [BC, T, P], F32, kind="ExternalOutput")
    pred_o = nc.dram_tensor("pred_o", [BC, T, 3], F32, kind="ExternalOutput")

    padimg = nc.dram_tensor("padimg", [CH * BC, PIMG, PIMG], F32)
    embih_d = nc.dram_tensor("embih_d", [2, 128, G4], BF)
    if debug:
        dbg_praw = nc.dram_tensor("dbg_praw", [128, 2, FLAT], F32,
                                  kind="ExternalOutput")
        dbg_emb = nc.dram_tensor("dbg_emb", [128, 2, NPATCH], F32,
                                 kind="ExternalOutput")
        dbg_init = nc.dram_tensor("dbg_init", [128, 19, BC], F32,
                                  kind="ExternalOutput")
        dbg_hc = nc.dram_tensor("dbg_hc", [BC, 2 * DEC_D], F32,
                                kind="ExternalOutput")

    import contextlib
    with tile.TileContext(nc) as tc, contextlib.ExitStack() as top:
        const = top.enter_context(tc.tile_pool(name="const", bufs=1))
        wts = top.enter_context(tc.tile_pool(name="wts", bufs=1))
        state = top.enter_context(tc.tile_pool(name="state", bufs=1))
        work = top.enter_context(tc.tile_pool(name="work", bufs=1))
        work2 = top.enter_context(tc.tile_pool(name="wrk2", bufs=2))
        emp = top.enter_context(tc.tile_pool(name="emp", bufs=2))

        def mm(out, lhsT, rhs, start, stop):
            nc.tensor.matmul(out, lhsT, rhs, start=start, stop=stop)

        # ---------- packed small consts ----------
        # packB (bf16): Wful [:,0:4], WfcsT [:,4:16], idb [:,16:144],
        #               ones row [0,144:272]
        packB = const.tile([128, 272], BF)
        nc.sync.dma_start(packB[:, 0:KD], Wful[:])
        nc.sync.dma_start(
            packB[:, KD:KD + KD * 3].rearrange("p (k n) -> p k n", n=3),
            Wfcs[:].rearrange("(k p) n -> p k n", p=128))
        idb = packB[:, 16:144]
        make_identity(nc, idb)
        nc.vector.memset(packB[0:1, 144:272], 1.0)
        ones1 = packB[0:1, 144:272]
        Wful_s = packB[:, 0:KD]
        Wfcs_s = packB[:, KD:KD + KD * 3].rearrange("p (k n) -> p k n", n=3)
        idf = const.tile([128, 128], F32)
        make_identity(nc, idf[:])
        if enable_bias["bpe"] or enable_bias["batt"]:
            bias_f = const.tile([128, 6], F32)
            nc.sync.dma_start(bias_f[:, 0:4], battT[:])
            nc.sync.dma_start(bias_f[:, 4:6], bpe[:])
        if enable_bias["bfb"] or enable_bias["bfc"]:
            bias_b = const.tile([1, G4 + 3], BF)
            nc.sync.dma_start(bias_b[0:1, 0:G4], W1b[:])
            nc.sync.dma_start(bias_b[0:1, G4:G4 + 3], bfcs[:])

        # resident group-1 weights, loaded early
        W1_s = wts.tile([128, KD, N1], BF)
        nc.sync.dma_start(W1_s[:], W1[:].rearrange("(k p) n -> p k n", p=128))

        # ---------- persistent state (touched early: allocation order) ----
        enc_pad = state.tile([128, 2 * BC, ENC_D], BF)
        att1T = state.tile([128, KD, BC * P], BF)
        # spack (bf16): abd [:,0:32] block-diag alpha, hT [:,32:48],
        #               vT [:,48:112]
        spack = state.tile([128, 112], BF)
        nc.vector.memset(spack[:], 0.0)
        abd = spack[:, 0:32].rearrange("p (j b) -> p j b", b=BC)
        hT = spack[:, 32:48].rearrange("p (k b) -> p k b", b=BC)
        vT = spack[:, 48:112].rearrange("p (k b) -> p k b", b=BC)
        hc2 = state.tile([BC, 2 * DEC_D], F32)
        h_s = hc2[:, 0:DEC_D]
        c_s = hc2[:, DEC_D:]
        nc.vector.memset(hc2[0:BC, 0:1], 0.0)

        # =========================== prologue ===========================
        with tc.tile_pool(name="proP", bufs=1, space="PSUM") as proP, \
             tc.tile_pool(name="proP2", bufs=2, space="PSUM") as proP2:

            # ---- enc load + cast into padded layout ----
            with tc.tile_pool(name="encl", bufs=2) as encl:
                for b in range(BC):
                    # zero pad rows 68:128 (32-aligned start; 64:68 is
                    # overwritten by the copy below)
                    nc.vector.memset(enc_pad[64:128, b * 2 + 1, :], 0.0)
                    for hf in range(2):
                        cnt = 128 if hf == 0 else P - 128
                        stg = encl.tile([128, ENC_D], F32, tag="encstg")
                        nc.sync.dma_start(stg[0:cnt, :],
                                          enc4[b, hf * 128:hf * 128 + cnt, :])
                        nc.vector.tensor_copy(enc_pad[0:cnt, b * 2 + hf, :],
                                              stg[0:cnt, :])

            with tc.tile_pool(name="proA", bufs=1) as proA:
                # ---- padded images + patch gather + patchesT ----
                patT = proA.tile([128, KF, NPATCH], BF, tag="patT")
                embT = proA.tile([128, 2, NPATCH], BF, tag="embT")
                initT = proA.tile([128, 19, BC], BF, tag="initT")
                mstg = proA.tile([128, KE, BC], F32, tag="mstg")
                with tc.tile_pool(name="proG", bufs=1) as proG:
                    zrow = proG.tile([128, PIMG], F32, tag="zrow")
                    nc.vector.memset(zrow[:], 0.0)
                    for bc in range(CH * BC):
                        nc.sync.dma_start(padimg[bc, 0:PAD, :], zrow[0:PAD, :])
                        for k4 in range(4):
                            nc.sync.dma_start(
                                padimg[bc, PAD + 128 * k4:PAD + 128 * (k4 + 1),
                                       0:PAD],
                                zrow[0:128, 0:PAD])
                    nc.sync.dma_start(padimg[:, PAD:PIMG, PAD:PIMG], imgs12[:])

                    poff_s = proG.tile([128, NPOFF], I32, tag="poff")
                    nc.sync.dma_start(poff_s[:], poff[:])
                    praw = proG.tile([128, 2, FLAT], F32, tag="praw")
                    NVALID = CH * BC * PIMG * PIMG - PATCH + 1
                    # HW indirect DMA: one entry per partition per
                    # instruction; coef=1 via a trailing fake dim, the
                    # 32-elem block size comes from the dest free size.
                    praw_rows = praw[:].rearrange("p h (e k) -> p (h e) k",
                                                  k=PATCH)
                    for j in range(NPOFF):
                        nc.gpsimd.indirect_dma_start(
                            out=praw_rows[:, j, :],
                            out_offset=None,
                            in_=bass.AP(tensor=padimg, offset=0,
                                        ap=[[1, NVALID], [1, 1]]),
                            in_offset=bass.IndirectOffsetOnAxis(
                                ap=poff_s[:, j:j + 1], axis=0),
                        )
                    if debug:
                        nc.sync.dma_start(dbg_praw[:], praw[:])
                    for hp in range(2):
                        for kc in range(KF):
                            tp = proP2.tile([128, 128], F32, tag="ps_t")
                            nc.tensor.transpose(
                                tp[:], praw[:, hp, kc * 128:(kc + 1) * 128],
                                idf[:])
                            nc.scalar.activation(
                                patT[:, kc, hp * 128:(hp + 1) * 128], tp[:],
                                AF.Copy)

                # ---- embT = W_pe @ patches.T ----
                with tc.tile_pool(name="proE", bufs=1) as proE:
                    WpeT_s = proE.tile([128, KF, EMB], BF, tag="wpet")
                    nc.sync.dma_start(
                        WpeT_s[:], WpeT[:].rearrange("(k p) n -> p k n", p=128))
                    for m in range(2):
                        ep = proP2.tile([128, NPATCH], F32, tag="ps_w")
                        for kc in range(KF):
                            mm(ep[:], WpeT_s[:, kc, m * 128:(m + 1) * 128],
                               patT[:, kc, :], kc == 0, kc == KF - 1)
                        if enable_bias["bpe"]:
                            nc.scalar.activation(embT[:, m, :], ep[:],
                                                 AF.Identity,
                                                 bias=bias_f[:, 4 + m:5 + m])
                        else:
                            nc.scalar.activation(embT[:, m, :], ep[:], AF.Copy)

                # ---- emb_ih -> DRAM (bf16) ----
                with tc.tile_pool(name="proI", bufs=1) as proI:
                    Wihe_s = proI.tile([128, 2, G4], BF, tag="wihe")
                    nc.sync.dma_start(
                        Wihe_s[:], Wihe[:].rearrange("(k p) n -> p k n", p=128))
                    bihh_s = proI.tile([1, G4], BF, tag="bihh")
                    nc.sync.dma_start(bihh_s[:], bihh[:])
                    for mt in range(2):
                        est = proI.tile([128, G4], BF, tag="est")
                        for q in range(4):
                            eo = proP2.tile([128, 512], F32, tag="ps_w")
                            for kc in range(2):
                                mm(eo[:], embT[:, kc, mt * 128:(mt + 1) * 128],
                                   Wihe_s[:, kc, q * 512:(q + 1) * 512],
                                   kc == 0, False)
                            mm(eo[:], ones1,
                               bihh_s[:, q * 512:(q + 1) * 512], False, True)
                            nc.scalar.activation(
                                est[:, q * 512:(q + 1) * 512], eo[:], AF.Copy)
                        nc.sync.dma_start(embih_d[mt, :, :], est[:])

                # initT pieces from embT
                nc.vector.memset(initT[:], 0.0)
                for hf in range(2):
                    nc.vector.tensor_copy(
                        initT[:, KE + hf, :],
                        embT[:, hf, :].rearrange("p (b t) -> p b t", t=T)[:, :, 0])
                nc.vector.memset(initT[0:1, 18, :], 1.0)

                if debug:
                    dstg = proA.tile([128, 2, NPATCH], F32, tag="dstg")
                    nc.vector.tensor_copy(dstg[:], embT[:])
                    nc.sync.dma_start(dbg_emb[:], dstg[:])

                # ---- att1 (k-halved encT rebuild, fp32 SBUF accum) ----
                with tc.tile_pool(name="proB", bufs=1) as proB:
                    att1A = proB.tile([128, KD, BC * P], F32, tag="att1A")
                    WencT_s = proB.tile([128, KE, ATT_D], BF, tag="wenct")
                    nc.sync.dma_start(
                        WencT_s[:], WencT[:].rearrange("(k p) n -> p k n", p=128))
                    for kh in range(2):
                        encTh = proB.tile([128, 8, 2 * BC * 128], BF,
                                          tag="encTh")
                        for j8 in range(2 * BC):
                            for kc in range(8):
                                tb = proP2.tile([128, 128], BF, tag="ps_t")
                                nc.tensor.transpose(
                                    tb[:],
                                    enc_pad[:, j8, (kh * 8 + kc) * 128:
                                            (kh * 8 + kc + 1) * 128], idb)
                                nc.scalar.activation(
                                    encTh[:, kc, j8 * 128:(j8 + 1) * 128],
                                    tb[:], AF.Copy)
                        for kc in range(8):
                            nc.vector.tensor_reduce(
                                mstg[:, kh * 8 + kc, :],
                                encTh[:, kc, :].rearrange(
                                    "p (b q) -> p b q", q=256),
                                axis=mybir.AxisListType.X, op=OP.add)
                        for m in range(KD):
                            # per-b slices padded to 256 so no matmul
                            # crosses a psum bank boundary
                            ap1 = proP.tile([128, BC, 256], F32, tag="ap1")
                            for b in range(BC):
                                for kc in range(8):
                                    mm(ap1[:, b, 0:P],
                                       WencT_s[:, kh * 8 + kc,
                                               m * 128:(m + 1) * 128],
                                       encTh[:, kc, b * 256:b * 256 + P],
                                       kc == 0, kc == 7)
                            src = ap1[:, :, 0:P]
                            dst = att1A[:, m, :].rearrange(
                                "p (b q) -> p b q", q=P)
                            if kh == 0:
                                nc.vector.tensor_copy(dst, src)
                            else:
                                nc.vector.tensor_tensor(
                                    out=dst, in0=dst, in1=src, op=OP.add)
                    for m in range(KD):
                        if enable_bias["batt"]:
                            nc.scalar.activation(att1T[:, m, :],
                                                 att1A[:, m, :], AF.Identity,
                                                 bias=bias_f[:, m:m + 1])
                        else:
                            nc.scalar.activation(att1T[:, m, :],
                                                 att1A[:, m, :], AF.Copy)

                # ---- mean -> initT, then h0/c0 ----
                nc.vector.tensor_scalar_mul(initT[:, 0:KE, :], mstg[:], 1.0 / P)
                with tc.tile_pool(name="proC", bufs=1) as proC:
                    Winit_s = proC.tile([128, 19, 2 * DEC_D], BF, tag="winit")
                    nc.sync.dma_start(
                        Winit_s[:], Winit[:].rearrange("(k p) n -> p k n", p=128))
                    for half in range(2):
                        hc = proP2.tile([BC, DEC_D], F32, tag="ps_w")
                        for kc in range(19):
                            mm(hc[:], initT[:, kc, :],
                               Winit_s[:, kc, half * DEC_D:(half + 1) * DEC_D],
                               kc == 0, kc == 18)
                        nc.vector.tensor_copy(
                            hc2[:, half * DEC_D:(half + 1) * DEC_D], hc[:])
                if debug:
                    istg = proA.tile([128, 19, BC], F32, tag="istg")
                    nc.vector.tensor_copy(istg[:], initT[:])
                    nc.sync.dma_start(dbg_init[:], istg[:])
                    nc.sync.dma_start(dbg_hc[:], hc2[:])

        # =========================== scan ===========================
        with tc.tile_pool(name="wtx", bufs=1) as wtx, \
             tc.tile_pool(name="psT", bufs=2, space="PSUM") as psT, \
             tc.tile_pool(name="psW", bufs=2, space="PSUM") as psW, \
             tc.tile_pool(name="psG", bufs=1, space="PSUM") as psG:

            Wx_s = wtx.tile([128, KE, G4], BF)
            nc.sync.dma_start(Wx_s[:], Wx[:].rearrange("(k p) n -> p k n", p=128))

            def make_hT(src):
                tp = psT.tile([128, KD * BC], F32, tag="ps_t")
                for kc in range(KD):
                    nc.tensor.transpose(tp[:, kc * BC:(kc + 1) * BC],
                                        src[:, kc * 128:(kc + 1) * 128],
                                        idf[0:BC, 0:BC])
                nc.vector.tensor_copy(hT[:].rearrange("p k b -> p (k b)"),
                                      tp[:])

            make_hT(h_s)

            for t in range(T):
                # emb_ih[t] prefetch
                eis = emp.tile([BC, G4], BF, tag="eis")
                nc.sync.dma_start(
                    eis[:],
                    bass.AP(tensor=embih_d, offset=t * G4,
                            ap=[[128 * G4, 2], [64 * G4, 2], [1, G4]]))

                # group 1 from hT: att2, gate-pre (sigmoid q-chunks)
                at2 = psW.tile([BC, ATT_D], F32, tag="ps_w")
                for kc in range(KD):
                    mm(at2[:], hT[:, kc, :], W1_s[:, kc, 0:ATT_D],
                       kc == 0, kc == KD - 1)
                gts = work.tile([BC, G4], BF, tag="gts")
                for q in range(4):
                    gp = psW.tile([BC, 512], F32, tag="ps_w")
                    for kc in range(KD):
                        mm(gp[:], hT[:, kc, :],
                           W1_s[:, kc, ATT_D + q * 512:ATT_D + (q + 1) * 512],
                           kc == 0, kc == KD - 1 and not enable_bias["bfb"])
                    if enable_bias["bfb"]:
                        mm(gp[:], ones1[0:1, 0:BC],
                           bias_b[0:1, q * 512:(q + 1) * 512], False, True)
                    nc.scalar.activation(gts[:, q * 512:(q + 1) * 512], gp[:],
                                         AF.Sigmoid)

                # att2T  (sc1: at2s [0:4, 0:512], at2T [:, 512:528])
                sc1 = work.tile([128, 528], F32, tag="sc1")
                at2s = sc1[0:BC, 0:ATT_D]
                at2T = sc1[:, ATT_D:ATT_D + KD * BC].rearrange(
                    "p (k b) -> p k b", b=BC)
                nc.vector.tensor_copy(at2s, at2[:])
                a2p = psT.tile([128, KD * BC], F32, tag="ps_t")
                for kc in range(KD):
                    nc.tensor.transpose(a2p[:, kc * BC:(kc + 1) * BC],
                                        at2s[:, kc * 128:(kc + 1) * 128],
                                        idf[0:BC, 0:BC])
                nc.vector.tensor_copy(
                    sc1[:, ATT_D:ATT_D + KD * BC], a2p[:])

                # relu(att1 + att2) per k-chunk, then scores into [1, 784]
                scp0 = psW.tile([1, HP], F32, tag="ps_w")
                scp1 = psW.tile([1, HP], F32, tag="ps_w")
                for kc in range(KD):
                    pre = work2.tile([128, BC * P], BF, tag="pre")
                    nc.vector.tensor_tensor(
                        out=pre[:].rearrange("p (b q) -> p b q", q=P),
                        in0=att1T[:, kc, :].rearrange("p (b q) -> p b q", q=P),
                        in1=at2T[:, kc, :].to_broadcast([128, BC, P]),
                        op=OP.add)
                    nc.scalar.activation(pre[:], pre[:], AF.Relu)
                    mm(scp0[:], Wful_s[:, kc:kc + 1], pre[:, 0:HP],
                       kc == 0, kc == KD - 1)
                    mm(scp1[:], Wful_s[:, kc:kc + 1], pre[:, HP:BC * P],
                       kc == 0, kc == KD - 1)

                # softmax scalars column-packed at base partition 0 (HW
                # requires equal base partitions for SB-SB tensor_tensor)
                # sm: al_f [0:784], exps [784:1568], sums [1568:1572],
                #     rcp [1572:1576], preds p0:4 [1576:1579]
                sm = work.tile([128, 1580], F32, tag="sm")
                al_f = sm[0:1, 0:BC * P]
                exps = sm[0:1, BC * P:2 * BC * P]
                sums = sm[0:1, 2 * BC * P:2 * BC * P + BC]
                rcp = sm[0:1, 2 * BC * P + BC:2 * BC * P + 2 * BC]
                for b in range(BC):
                    src = scp0 if b < 2 else scp1
                    nc.scalar.activation(
                        exps[0:1, b * P:(b + 1) * P],
                        src[0:1, (b % 2) * P:((b % 2) + 1) * P], AF.Exp,
                        accum_out=sums[0:1, b:b + 1])
                nc.vector.reciprocal(rcp, sums)
                nc.vector.tensor_tensor(
                    out=al_f.rearrange("o (b q) -> o b q", q=P),
                    in0=exps.rearrange("o (b q) -> o b q", q=P),
                    in1=rcp.to_broadcast([1, BC, P]), op=OP.mult)
                nc.sync.dma_start(alph_o[:, t, :], al_f)

                # block-diag alpha (196 = 128 + 68 per lane)
                alp = psT.tile([128, 2 * BC], F32, tag="ps_t")
                for b in range(BC):
                    for hf in range(2):
                        cnt = 128 if hf == 0 else P - 128
                        nc.tensor.transpose(
                            alp[0:cnt, b * 2 + hf:b * 2 + hf + 1],
                            al_f[0:1, b * P + hf * 128:b * P + hf * 128 + cnt],
                            idf[0:1, 0:1])
                for b in range(BC):
                    for hf in range(2):
                        cnt = 128 if hf == 0 else P - 128
                        nc.vector.tensor_copy(
                            abd[0:cnt, b * 2 + hf, b:b + 1],
                            alp[0:cnt, b * 2 + hf:b * 2 + hf + 1])

                # awe + v = gate * awe   (v in bf16)
                v_s = work.tile([BC, G4], BF, tag="v_s")
                for q in range(4):
                    awp = psW.tile([BC, 512], F32, tag="ps_w")
                    for j in range(2 * BC):
                        mm(awp[:], abd[:, j, :],
                           enc_pad[:, j, q * 512:(q + 1) * 512],
                           j == 0, j == 2 * BC - 1)
                    nc.vector.tensor_tensor(out=v_s[:, q * 512:(q + 1) * 512],
                                            in0=gts[:, q * 512:(q + 1) * 512],
                                            in1=awp[:], op=OP.mult)

                # vT (bf16 transposes)
                vtp = psT.tile([128, KE * BC], BF, tag="ps_t")
                for kc in range(KE):
                    nc.tensor.transpose(vtp[:, kc * BC:(kc + 1) * BC],
                                        v_s[:, kc * 128:(kc + 1) * 128],
                                        idb[0:BC, 0:BC])
                nc.vector.tensor_copy(spack[:, 48:112], vtp[:])

                # gates = h @ Whh.T + v @ Wx.T  (one PSUM accum group per bank)
                gate = psG.tile([BC, G4], F32, tag="gates")
                for q in range(4):
                    for kc in range(KD):
                        mm(gate[:, q * 512:(q + 1) * 512], hT[:, kc, :],
                           W1_s[:, kc, ATT_D + G4 + q * 512:
                                ATT_D + G4 + (q + 1) * 512],
                           kc == 0, False)
                for q in range(4):
                    for kc in range(KE):
                        mm(gate[:, q * 512:(q + 1) * 512], vT[:, kc, :],
                           Wx_s[:, kc, q * 512:(q + 1) * 512],
                           False, kc == KE - 1)

                # gfull = gates + emb_ih[t]; LSTM pointwise (in-place sig/tanh)
                gfull = work2.tile([BC, G4], F32, tag="gfull")
                nc.vector.tensor_tensor(out=gfull[:], in0=gate[:], in1=eis[:],
                                        op=OP.add)
                for q, f in ((0, AF.Sigmoid), (1, AF.Sigmoid), (2, AF.Tanh),
                             (3, AF.Sigmoid)):
                    nc.scalar.activation(gfull[:, q * 512:(q + 1) * 512],
                                         gfull[:, q * 512:(q + 1) * 512], f)
                t12 = work.tile([BC, 2 * DEC_D], F32, tag="t12")
                t1 = t12[:, 0:DEC_D]
                t2 = t12[:, DEC_D:]
                nc.vector.tensor_tensor(out=t1, in0=gfull[:, 512:1024],
                                        in1=c_s, op=OP.mult)
                nc.vector.tensor_tensor(out=t2, in0=gfull[:, 0:512],
                                        in1=gfull[:, 1024:1536], op=OP.mult)
                nc.vector.tensor_tensor(out=c_s, in0=t1, in1=t2, op=OP.add)
                nc.scalar.activation(t1, c_s, AF.Tanh)
                nc.vector.tensor_tensor(out=h_s, in0=gfull[:, 1536:2048],
                                        in1=t1, op=OP.mult)

                # preds from new h
                make_hT(h_s)
                prp = psW.tile([BC, 3], F32, tag="ps_w")
                for kc in range(KD):
                    mm(prp[:], hT[:, kc, :], Wfcs_s[:, kc, :],
                       kc == 0, kc == KD - 1 and not enable_bias["bfc"])
                if enable_bias["bfc"]:
                    mm(prp[:], ones1[0:1, 0:BC], bias_b[0:1, G4:G4 + 3],
                       False, True)
                preds = sm[0:BC, 2 * BC * P + 2 * BC:2 * BC * P + 2 * BC + 3]
                nc.vector.tensor_copy(preds[:, 0:2], prp[:, 0:2])
                nc.scalar.activation(preds[:, 2:3], prp[:, 2:3], AF.Sigmoid)
                nc.sync.dma_start(pred_o[:, t, :], preds)

    nc.compile()
    return nc


_NC_CACHE = {}


def _get_nc(enable_bias):
    key = tuple(sorted(enable_bias.items()))
    if key not in _NC_CACHE:
        _NC_CACHE[key] = build_program(enable_bias)
    return _NC_CACHE[key]


def host_prepare(inputs):
    lengths_full = np.asarray(inputs["caption_lengths"])[:, 0]
    sort_ind = np.argsort(-lengths_full, kind="stable").astype(np.int32)
    lengths = lengths_full[sort_ind]

    enc = np.asarray(inputs["encoder_out"], np.float32)[sort_ind].reshape(
        B, P, ENC_D)
    imgs = np.asarray(inputs["imgs"], np.float32)[sort_ind]
    seq = np.asarray(inputs["sequence"])[sort_ind]
    seq_off = np.asarray(inputs["sequence_offset"], np.float32)[sort_ind]

    g = lambda k: np.asarray(inputs[k], np.float32)
    bf = lambda x: np.ascontiguousarray(x, np.float32).astype(ml_dtypes.bfloat16)

    W_pe, b_pe = g("W_pe"), g("b_pe")
    W_enc_att, b_enc_att = g("W_enc_att"), g("b_enc_att")
    W_dec_att, b_dec_att = g("W_dec_att"), g("b_dec_att")
    w_full = g("w_full")
    W_fb, b_fb = g("W_fb"), g("b_fb")
    W_ih, b_ih = g("W_ih"), g("b_ih")
    W_hh, b_hh = g("W_hh"), g("b_hh")
    W_init_h, b_init_h = g("W_init_h"), g("b_init_h")
    W_init_c, b_init_c = g("W_init_c"), g("b_init_c")
    W_fc, b_fc = g("W_fc"), g("b_fc")
    W_stop, b_stop = g("W_stop"), g("b_stop")

    enable_bias = dict(
        bpe=bool(np.any(b_pe)),
        batt=bool(np.any(b_enc_att) or np.any(b_dec_att)),
        bfb=bool(np.any(b_fb)),
        bfc=bool(np.any(b_fc) or np.any(b_stop)),
    )

    shared = dict(
        W1=bf(np.concatenate([W_dec_att.T, W_fb.T, W_hh.T], axis=1)),
        W1b=bf(b_fb[None, :]),
        Wx=bf(W_ih[:, :ENC_D].T),
        WpeT=bf(W_pe.T),
        bpe=np.ascontiguousarray(b_pe.reshape(2, 128).T, np.float32),
        WencT=bf(W_enc_att.T),
        battT=np.ascontiguousarray(
            (b_enc_att + b_dec_att).reshape(4, 128).T, np.float32),
        Winit=bf(np.concatenate([
            np.concatenate([W_init_h, W_init_c], axis=0).T,
            np.concatenate([b_init_h, b_init_c])[None, :],
            np.zeros((19 * 128 - 2305, 2 * DEC_D), np.float32)], axis=0)),
        Wihe=bf(W_ih[:, ENC_D:].T),
        bihh=bf((b_ih + b_hh)[None, :]),
        Wfcs=bf(np.concatenate([W_fc, W_stop], axis=0).T),
        bfcs=bf(np.concatenate([b_fc, b_stop])[None, :]),
        Wful=bf(w_full.reshape(4, 128).T),
    )

    in_maps = []
    for c in range(NCORES):
        bs = slice(c * BC, (c + 1) * BC)
        imgs_c = imgs[bs].reshape(CH * BC, IMG, IMG)
        seq_c = seq[bs]
        x = seq_c[..., 0].astype(np.int64)
        y = seq_c[..., 1].astype(np.int64)
        b_idx = np.arange(BC)[:, None, None, None]
        c_idx = np.arange(CH)[None, None, :, None]
        r_idx = np.arange(PATCH)[None, None, None, :]
        off = ((b_idx * CH + c_idx) * PIMG + y[:, :, None, None] + r_idx) \
            * PIMG + x[:, :, None, None]
        off = off.reshape(BC * T, CH * PATCH)
        poff_np = np.zeros((128, NPOFF), np.int32)
        for patch in range(NPATCH):
            poff_np[patch % 128, (patch // 128) * 96:(patch // 128 + 1) * 96] \
                = off[patch]
        m = dict(shared)
        m.update(
            imgs12=np.ascontiguousarray(imgs_c, np.float32),
            poff=poff_np,
            enc4=np.ascontiguousarray(enc[bs], np.float32),
        )
        in_maps.append(m)

    host_out = dict(sort_ind=sort_ind, lengths=lengths, seq_off=seq_off,
                    enable_bias=enable_bias)
    return in_maps, host_out


LAST_RESULT = None


def kernel(**inputs):
    global LAST_RESULT
    import os
    in_maps, host_out = host_prepare(inputs)
    nc = _get_nc(host_out["enable_bias"])
    res = bass_utils.run_bass_kernel_spmd(
        nc, in_maps, core_ids=list(range(NCORES)),
        trace=os.environ.get("KERNEL_TRACE") == "1")
    LAST_RESULT = res
    preds = np.concatenate([r["pred_o"] for r in res.results], axis=0)
    alphas = np.concatenate([r["alph_o"] for r in res.results], axis=0)
    # host-side length masking (exact zeroing, as in the reference)
    mask = (np.arange(T)[None, :] < host_out["lengths"][:, None])
    preds = preds * mask[:, :, None]
    alphas = alphas * mask[:, :, None]
    predictions = np.ascontiguousarray(preds[:, :, 0:2])
    predictions_stop = np.ascontiguousarray(preds[:, :, 2:3])
    return (predictions, predictions_stop, host_out["seq_off"],
            host_out["lengths"].astype(np.int32), alphas,
            host_out["sort_ind"])
